# revision 7
# baseline (speedup 1.0000x reference)
"""Correlation3D Trainium2 kernel (8 NeuronCores, SPMD).

Sharding: core c -> batch c//4, query rows [(c%4)*2048, (c%4)*2048+2048).
xyz2/feat2 replicated per batch group. All compute on device.

Per core:
  - pyramid kNN (k=3) between xyz2 levels -> effective feat2 levels
    (cost-volume pyramid is linear in feat2, so gather+sum feat2 columns)
  - per query tile (128 rows) and level: distances via bf16 3-term-split
    matmul (K=21, ~fp32 exact), per-chunk top8 (DVE max) + index recovery
    (max_index), merge to top-16; gather f2/u columns (gpsimd ap_gather);
    correlation + both MLP layers + k-sum + final 64x64 matmul all as
    PSUM-accumulated matmuls.
"""

import numpy as np
import ml_dtypes
from contextlib import ExitStack

import concourse.bass as bass
import concourse.tile as tile
from concourse import bacc, mybir, library_config
from concourse.bass_utils import run_bass_kernel_spmd

bf16 = ml_dtypes.bfloat16
FP = mybir.dt.float32
BF = mybir.dt.bfloat16
U32 = mybir.dt.uint32
I16 = mybir.dt.int16
AF = mybir.ActivationFunctionType
ALU = mybir.AluOpType

B, N1, CIN, C = 2, 8192, 128, 64
NS = [8192, 2048, 512, 128]
KC, KP = 16, 3
NCORES = 8
RPC = N1 * B // NCORES  # 2048
NTILES = RPC // 128     # 16
NEG = -3.0e38

TERMS = [(0, 0), (0, 1), (1, 0), (0, 2), (2, 0), (1, 1)]


def _split3(x):
    x = np.asarray(x, np.float64)
    t0 = x.astype(bf16).astype(np.float64)
    t1 = (x - t0).astype(bf16).astype(np.float64)
    t2 = (x - t0 - t1).astype(bf16).astype(np.float64)
    return [t.astype(np.float32) for t in (t0, t1, t2)]


def _dist_moving(xyz):
    p2 = (np.asarray(xyz, np.float64) ** 2).sum(0)
    ps = [_split3(xyz[d]) for d in range(3)]
    rows = []
    for d in range(3):
        for (_, j) in TERMS:
            rows.append(ps[d][j])
    rows += _split3(p2)
    return np.stack(rows).astype(bf16)  # [21, N]


def _dist_stationary(xyz):
    qs = [_split3(xyz[d]) for d in range(3)]
    rows = []
    for d in range(3):
        for (i, _) in TERMS:
            rows.append(2.0 * qs[d][i])
    one = -np.ones(xyz.shape[1], np.float32)
    rows += [one, one, one]
    return np.stack(rows).astype(bf16)  # [21, N]


def _a1_stat(w1, b1):
    w1h = w1[:, :3].astype(bf16).astype(np.float32)
    w1l = (w1[:, :3] - w1h).astype(bf16).astype(np.float32)
    b1h = b1.astype(bf16).astype(np.float32)
    b1l = (b1 - b1h).astype(bf16).astype(np.float32)
    stat = np.zeros((11, 16), np.float32)
    for d in range(3):
        stat[d] = -w1h[:, d]
        stat[3 + d] = -w1h[:, d]
        stat[6 + d] = -w1l[:, d]
    stat[9] = b1h
    stat[10] = b1l
    return stat.astype(bf16)


def _a1_moving(xyz1own):
    qs = [_split3(xyz1own[d]) for d in range(3)]
    n = xyz1own.shape[1]
    rows = [qs[0][0], qs[1][0], qs[2][0]]
    rows += [(qs[d][1] + qs[d][2]).astype(bf16).astype(np.float32) for d in range(3)]
    rows += [qs[0][0], qs[1][0], qs[2][0]]
    one = np.ones(n, np.float32)
    rows += [one, one]
    return np.stack(rows).astype(bf16)  # [11, N]


def _split2T(w):
    h = w.astype(bf16).astype(np.float32)
    l = (w - h).astype(bf16)
    return np.concatenate([h.astype(bf16).T, l.T], axis=0)  # [2K, M]


def build_host_inputs(inputs):
    xyz1 = np.asarray(inputs['xyz1'], np.float32)
    lv = [np.asarray(inputs[f'xyz2_{i}'], np.float32) for i in range(4)]
    feat1 = np.asarray(inputs['feat1'], np.float32)
    feat2 = np.asarray(inputs['feat2'], np.float32)
    w1 = np.asarray(inputs['w1'], np.float32); b1 = np.asarray(inputs['b1'], np.float32)
    w2 = np.asarray(inputs['w2'], np.float32); b2 = np.asarray(inputs['b2'], np.float32)
    wm = np.asarray(inputs['wm'], np.float32); bm = np.asarray(inputs['bm'], np.float32)

    per_batch = []
    for b in range(B):
        pb = {'feat2': np.ascontiguousarray(feat2[b])}
        for l in range(4):
            pb[f'mov{l}'] = np.ascontiguousarray(_dist_moving(lv[l][b]))
            pb[f'xyzr{l}'] = np.ascontiguousarray(
                np.concatenate([lv[l][b], np.zeros((1, NS[l]), np.float32)]))
        for l in range(1, 4):
            pb[f'pstat{l}'] = np.ascontiguousarray(_dist_stationary(lv[l][b]))
        per_batch.append(pb)

    common = {
        'w1t3': np.ascontiguousarray(w1[:, :3].T),
        'a1stat': _a1_stat(w1, b1),
        'w2t2': np.ascontiguousarray(_split2T(w2)),
        'b1c': np.ascontiguousarray(b1.reshape(16, 1)),
        'b2c': np.ascontiguousarray(b2.reshape(16, 1)),
        'bmc': np.ascontiguousarray(bm.reshape(64, 1)),
        'ident': np.eye(128, dtype=np.float32),
        'identb': np.eye(16, dtype=bf16),
        'coff': np.tile(np.repeat(np.arange(16, dtype=np.float32) * 512, 8)[None, :],
                        (128, 1)),
    }
    for l in range(4):
        common[f'wmt2_{l}'] = np.ascontiguousarray(_split2T(wm[:, 16 * l:16 * l + 16]))
        common[f'w14g{l}'] = np.ascontiguousarray(
            np.tile((w1[:, 3] / (3.0 ** l)).astype(np.float32)[None, :],
                    (128, 1)).astype(bf16))

    in_maps = []
    for core in range(NCORES):
        b = core // 4
        r0 = (core % 4) * RPC
        sl = slice(r0, r0 + RPC)
        m = dict(common)
        m.update(per_batch[b])
        m['qstat'] = np.ascontiguousarray(_dist_stationary(xyz1[b][:, sl]))
        m['a1mov'] = np.ascontiguousarray(_a1_moving(xyz1[b][:, sl]))
        m['feat1s'] = np.ascontiguousarray(feat1[b][:, sl] / np.float32(CIN))
        in_maps.append(m)
    return in_maps


# ---------------------------------------------------------------------------

def build_program():
    nc = bacc.Bacc("TRN2", target_bir_lowering=False, debug=False,
                   num_devices=NCORES)

    def din(name, shape, dt):
        return nc.dram_tensor(name, list(shape), dt, kind="ExternalInput").ap()

    qstatD = din('qstat', (21, RPC), BF)
    a1movD = din('a1mov', (11, RPC), BF)
    feat1D = din('feat1s', (128, RPC), FP)
    movD = [din(f'mov{l}', (21, NS[l]), BF) for l in range(4)]
    xyzrD = [din(f'xyzr{l}', (4, NS[l]), FP) for l in range(4)]
    pstatD = {l: din(f'pstat{l}', (21, NS[l]), BF) for l in range(1, 4)}
    feat2D = din('feat2', (128, 8192), FP)
    w1t3D = din('w1t3', (3, 16), FP)
    a1statD = din('a1stat', (11, 16), BF)
    w2t2D = din('w2t2', (32, 16), BF)
    wmt2D = [din(f'wmt2_{l}', (32, 64), BF) for l in range(4)]
    w14gD = [din(f'w14g{l}', (128, 16), BF) for l in range(4)]
    b1cD = din('b1c', (16, 1), FP)
    b2cD = din('b2c', (16, 1), FP)
    bmcD = din('bmc', (64, 1), FP)
    identD = din('ident', (128, 128), FP)
    identbD = din('identb', (16, 16), BF)
    coffD = din('coff', (128, 128), FP)

    outD = nc.dram_tensor('out', [C, RPC], FP, kind="ExternalOutput").ap()
    # DRAM scratch for partition-fold reshuffles of pyramid indices
    pfoldD = [nc.dram_tensor(f'pfold{l}', [NS[l] * KP], FP).ap() for l in range(1, 4)]

    with tile.TileContext(nc) as tc, ExitStack() as ctx:
        nc.gpsimd.load_library(library_config.ap_gather)
        cpool = ctx.enter_context(tc.tile_pool(name="const", bufs=1))
        lvl = ctx.enter_context(tc.tile_pool(name="lvl", bufs=1))
        sel = ctx.enter_context(tc.tile_pool(name="sel", bufs=1))
        mlp = ctx.enter_context(tc.tile_pool(name="mlp", bufs=1))
        gat = ctx.enter_context(tc.tile_pool(name="gat", bufs=1))
        psd = ctx.enter_context(tc.tile_pool(name="psd", bufs=2, space="PSUM"))
        psm = ctx.enter_context(tc.tile_pool(name="psm", bufs=1, space="PSUM"))
        psacc = ctx.enter_context(tc.tile_pool(name="psacc", bufs=1, space="PSUM"))
        pstr = ctx.enter_context(tc.tile_pool(name="pstr", bufs=1, space="PSUM"))

        def load(pool, ap, tag):
            t = pool.tile(list(ap.shape), ap.dtype, tag=tag)
            nc.sync.dma_start(t[:], ap[:])
            return t

        ident = load(cpool, identD, 'ident')
        identb = load(cpool, identbD, 'identb')
        coff = load(cpool, coffD, 'coff')
        qstat = load(cpool, qstatD, 'qstat')
        a1mov = load(cpool, a1movD, 'a1mov')
        feat1 = load(cpool, feat1D, 'feat1')
        a1stat = load(cpool, a1statD, 'a1stat')
        w2t2 = load(cpool, w2t2D, 'w2t2')
        wmt2 = [load(cpool, wmt2D[l], f'wmt2{l}') for l in range(4)]
        w14g = [load(cpool, w14gD[l], f'w14g{l}') for l in range(4)]
        b1c = load(cpool, b1cD, 'b1c')
        b2c = load(cpool, b2cD, 'b2c')
        bmc = load(cpool, bmcD, 'bmc')
        w1t3 = load(cpool, w1t3D, 'w1t3')
        mov = [None] + [load(cpool, movD[l], f'mov{l}') for l in range(1, 4)]
        feat2 = load(cpool, feat2D, 'feat2')
        movch = ctx.enter_context(tc.tile_pool(name="movch", bufs=3))

        def dist_mm(stat_ap, lv_i, c, n=512):
            if lv_i == 0:
                mv = movch.tile([21, 512], BF, tag="movch")
                nc.sync.dma_start(mv[:, 0:n], movD[0][:, 512 * c:512 * c + n])
                mva = mv[:, 0:n]
            else:
                mva = mov[lv_i][:, 512 * c:512 * c + n]
            ps = psd.tile([128, 512], FP, tag="dps")
            nc.tensor.matmul(ps[:, 0:n], stat_ap, mva, start=True, stop=True)
            return ps

        def sel_chunked(n_chunk, k_out, mm_fn, nmax):
            v8 = sel.tile([128, n_chunk * 8], FP, tag="v8cat")
            i8 = sel.tile([128, n_chunk * 8], U32, tag="i8cat")
            for c in range(n_chunk):
                ps = mm_fn(c)
                dch = movch.tile([128, 512], FP, tag="dchunk")
                nc.scalar.activation(dch[:], ps[:], AF.Copy)
                nc.vector.max(v8[:, 8 * c:8 * c + 8], dch[:])
                nc.vector.max_index(i8[:, 8 * c:8 * c + 8], v8[:, 8 * c:8 * c + 8],
                                    dch[:])
            gif = sel.tile([128, n_chunk * 8], FP, tag="gif")
            nc.vector.tensor_copy(gif[:], i8[:])
            nc.vector.tensor_add(gif[:], gif[:], coff[:, :n_chunk * 8])
            w = sel.tile([128, 16], FP, tag="selw")
            nc.vector.max(w[:, 0:8], v8[:])
            if k_out > 8:
                v8b = sel.tile([128, n_chunk * 8], FP, tag="v8cat2")
                nc.vector.match_replace(v8b[:], w[:, 0:8], v8[:], NEG)
                nc.vector.max(w[:, 8:16], v8b[:])
            g = sel.tile([128, k_out], FP, tag="selg")
            dummy = sel.tile([128, n_chunk * 8], FP, tag="seldummy")
            for j in range(k_out):
                nc.vector.scalar_tensor_tensor(
                    dummy[:], v8[:], w[:, j:j + 1], gif[:],
                    ALU.is_equal, ALU.mult, accum_out=g[:, j:j + 1])
            # tie-safety clamp (duplicate values sum their indices)
            nc.vector.tensor_scalar_min(g[:], g[:], float(nmax - 1))
            return g

        def sel_direct(n, k_out, mm_fn, n_chunk):
            row = sel.tile([128, 2048], FP, tag="drow")
            for c in range(n_chunk):
                ps = mm_fn(c)
                nn = min(512, n)
                nc.scalar.activation(row[:, 512 * c:512 * c + nn], ps[:, 0:nn], AF.Copy)
            w = sel.tile([128, 16], FP, tag="selw")
            iu = sel.tile([128, 16], U32, tag="seliu")
            nc.vector.max(w[:, 0:8], row[:, 0:n])
            nc.vector.max_index(iu[:, 0:8], w[:, 0:8], row[:, 0:n])
            if k_out > 8:
                row2 = sel.tile([128, 2048], FP, tag="drow2")
                nc.vector.match_replace(row2[:, 0:n], w[:, 0:8], row[:, 0:n], NEG)
                nc.vector.max(w[:, 8:16], row2[:, 0:n])
                nc.vector.max_index(iu[:, 8:16], w[:, 8:16], row2[:, 0:n])
            g = sel.tile([128, k_out], FP, tag="selg")
            nc.vector.tensor_copy(g[:], iu[:, 0:k_out])
            return g

        def wrap16(g, k):
            """[128, k] f32 -> wrapped int16 idx tile [128, 128] (8 replicas)."""
            pt = pstr.tile([16, 512], FP, tag="trps")
            nc.tensor.transpose(pt[0:k, 0:128], g[:], ident[:])
            wi = sel.tile([128, 128], I16, tag="wrapidx")
            nc.scalar.activation(wi[0:16, :], pt[0:16, 0:128], AF.Copy)
            for r in range(1, 8):
                nc.sync.dma_start(wi[16 * r:16 * (r + 1), :], wi[0:16, :])
            return wi

        # ---------------- pyramid ----------------
        f2raw = [feat2]
        with ExitStack() as pctx:
            ppool = pctx.enter_context(tc.tile_pool(name="pyr", bufs=1))
            pscr = pctx.enter_context(tc.tile_pool(name="pyrs", bufs=1))
            pstat = {l: load(ppool, pstatD[l], f'pstat{l}') for l in range(1, 4)}

            for l in range(1, 4):
                nq, ncand = NS[l], NS[l - 1]
                for t in range(nq // 128):
                    st = pstat[l][:, 128 * t:128 * (t + 1)]
                    if ncand > 2048:
                        g3 = sel_chunked(ncand // 512, KP,
                                         lambda c: dist_mm(st, l - 1, c), ncand)
                    else:
                        g3 = sel_direct(ncand, KP,
                                        lambda c: dist_mm(st, l - 1, c),
                                        max(1, ncand // 512))
                    # fold [128, 3] -> DRAM linear i = q*3+k
                    nc.sync.dma_start(
                        pfoldD[l - 1][384 * t:384 * (t + 1)].rearrange(
                            "(p f) -> p f", p=128), g3[:])
                # wrapped int16 list: element i at [i%16, i//16]
                nidx = nq * KP
                wrapf = pscr.tile([16, nidx // 16], FP, tag="pwrapf")
                nc.sync.dma_start(wrapf[:],
                                  pfoldD[l - 1][:].rearrange("(f p) -> p f", p=16))
                wrap = ppool.tile([128, nidx // 16], I16, tag=f"pwrap{l}")
                nc.vector.tensor_copy(wrap[0:16, :], wrapf[:])
                for r in range(1, 8):
                    nc.sync.dma_start(wrap[16 * r:16 * (r + 1), :], wrap[0:16, :])
                gath = pscr.tile([128, nidx], FP, tag="pgath")
                nc.gpsimd.ap_gather(gath[:, 0:nidx], f2raw[l - 1][:].unsqueeze(2),
                                    wrap[:], channels=128, num_elems=ncand, d=1,
                                    num_idxs=nidx)
                f2n = lvl.tile([128, nq], FP, tag=f"f2raw{l}")
                nc.vector.tensor_reduce(
                    f2n[:], gath[:, 0:nidx].rearrange("p (n k) -> p n k", k=KP),
                    mybir.AxisListType.X, ALU.add)
                f2raw.append(f2n)

            # u tiles: u = w1[:, :3] @ xyz2 ; stored [16, N, 2] bf16 (hi, lo)
            ut = []
            for l in range(4):
                u = lvl.tile([16, NS[l], 2], BF, tag=f"u{l}")
                for c in range(max(1, NS[l] // 512)):
                    nn = min(512, NS[l])
                    csl = slice(512 * c, 512 * c + nn)
                    xch = pscr.tile([3, 512], FP, tag="xch")
                    nc.sync.dma_start(xch[:, 0:nn], xyzrD[l][0:3, csl])
                    ups = pstr.tile([16, 512], FP, tag="trps")
                    nc.tensor.matmul(ups[:, 0:nn], w1t3[:], xch[:, 0:nn],
                                     start=True, stop=True)
                    uf = pscr.tile([16, 512], FP, tag="uf")
                    nc.scalar.activation(uf[:, 0:nn], ups[:, 0:nn], AF.Copy)
                    uhb = u[:, csl, 0:1].rearrange("p a b -> p (a b)")
                    nc.scalar.activation(uhb, uf[:, 0:nn], AF.Copy)
                    uh32 = pscr.tile([16, 512], FP, tag="uh32")
                    nc.vector.tensor_copy(uh32[:, 0:nn], uhb)
                    nc.vector.tensor_sub(u[:, csl, 1:2].rearrange("p a b -> p (a b)"),
                                         uf[:, 0:nn], uh32[:, 0:nn])
                ut.append(u)

        # ---------------- main loop ----------------
        for t in range(NTILES):
            qs = qstat[:, 128 * t:128 * (t + 1)]
            acc = psacc.tile([64, 128], FP, tag="accps")
            nmm = [0]
            for l in range(4):
                n = NS[l]
                if n > 2048:
                    g16 = sel_chunked(n // 512, KC,
                                      lambda c: dist_mm(qs, l, c), n)
                else:
                    g16 = sel_direct(n, KC,
                                     lambda c: dist_mm(qs, l, c, n=min(512, n)),
                                     max(1, n // 512))
                wi = wrap16(g16, KC)
                f2g = gat.tile([128, 2048], FP, tag="f2gt")
                nc.gpsimd.ap_gather(f2g[:], f2raw[l][:].unsqueeze(2), wi[:],
                                    channels=128, num_elems=n, d=1, num_idxs=2048)
                ug = gat.tile([16, 2048, 2], BF, tag="ugt")
                nc.gpsimd.ap_gather(ug[:], ut[l][:], wi[0:16, :],
                                    channels=16, num_elems=n, d=2, num_idxs=2048)
                G = gat.tile([128, 2048], BF, tag="Gt")
                f1v = feat1[:, 128 * t:128 * (t + 1)].unsqueeze(2) \
                    .broadcast_to([128, 128, 16])
                nc.vector.tensor_mul(G[:].rearrange("p (q k) -> p q k", k=16),
                                     f2g[:].rearrange("p (q k) -> p q k", k=16), f1v)
                a1v = a1mov[:, 128 * t:128 * (t + 1)].unsqueeze(2) \
                    .broadcast_to([11, 128, 16])
                pre = psm.tile([16, 2048], FP, tag="mlpps")
                uge = ug[:, :, 0:1].rearrange("p a b -> p (a b)")
                ugo = ug[:, :, 1:2].rearrange("p a b -> p (a b)")
                for c in range(4):
                    csl = slice(512 * c, 512 * (c + 1))
                    qsl = slice(32 * c, 32 * (c + 1))
                    nc.tensor.matmul(pre[:, csl], w14g[l][:], G[:, csl],
                                     start=True, stop=False)
                    nc.tensor.matmul(pre[:, csl], identb[:], uge[:, csl],
                                     start=False, stop=False)
                    nc.tensor.matmul(pre[:, csl], identb[:], ugo[:, csl],
                                     start=False, stop=False)
                    nc.tensor.matmul(pre[:, csl], a1stat[:], a1v[:, qsl, :],
                                     start=False, stop=True)
                h1 = mlp.tile([32, 2048], BF, tag="h1t")
                nc.scalar.activation(h1[0:16, :], pre[:], AF.Relu)
                nc.sync.dma_start(h1[16:32, :], h1[0:16, :])
                h2p = psm.tile([16, 2048], FP, tag="mlpps")
                for c in range(4):
                    csl = slice(512 * c, 512 * (c + 1))
                    nc.tensor.matmul(h2p[:, csl], w2t2[:], h1[:, csl],
                                     start=True, stop=True)
                h2 = mlp.tile([32, 2048], BF, tag="h2t")
                nc.scalar.activation(h2[0:16, :], h2p[:], AF.Relu, bias=b2c[:])
                nc.sync.dma_start(h2[16:32, :], h2[0:16, :])
                h2v = h2[:].rearrange("p (q k) -> p k q", k=16)
                for k in range(16):
                    nmm[0] += 1
                    nc.tensor.matmul(acc[:], wmt2[l][:], h2v[:, k:k + 1, :].squeeze(1),
                                     start=(nmm[0] == 1), stop=(nmm[0] == 64))
            ot = mlp.tile([64, 128], FP, tag="outt")
            nc.scalar.activation(ot[:], acc[:], AF.Relu, bias=bmc[:])
            nc.sync.dma_start(outD[:, 128 * t:128 * (t + 1)], ot[:])

    nc.compile()
    return nc


_NC_CACHE = None


def kernel(**inputs):
    global _NC_CACHE
    in_maps = build_host_inputs(inputs)
    if _NC_CACHE is None:
        _NC_CACHE = build_program()
    res = run_bass_kernel_spmd(_NC_CACHE, in_maps, list(range(NCORES))).results
    out = np.zeros((B, C, N1), np.float32)
    for core in range(NCORES):
        b = core // 4
        r0 = (core % 4) * RPC
        out[b][:, r0:r0 + RPC] = res[core]['out']
    return out
